# revision 21
# baseline (speedup 1.0000x reference)
"""Trainium2 kernel for nn_Basic3DBlock (sparse 3D conv + sync BN + ReLU).

Strategy: the neighbor map was generated from a hidden embedding of the N
voxels into a dense 3D grid (27-point stencil). The host reconstructs exact
relative coordinates from neighbor_idx by BFS over the 26 directed neighbor
relations (components packed into a zero-padded box), turning the sparse
gather-conv into a DENSE 27-tap stencil — no indirect DMA at all.

Device layout: one x-plane of the box is stored channel-major in 8 blocks
stacked on the partition axis with an in-plane halo: tile [128, COLS],
partition (b*16+c) col v = channel c of plane cell (b*BLKP + v - H). All 9
in-plane shifts are column offsets of that tile; the x-shifts pick one of 3
resident plane tiles. Weights become 27 block-diagonal [128,128] bf16
matrices; 27 accumulating matmuls per [128,512] PSUM tile compute conv for
4096 cells. A mask (1 at occupied cells) zeroes garbage at empty cells and
feeds masked sum/sumsq row-reductions for sync BN (free on the scalar
engine's activation accumulator). Everything runs in ONE NEFF launch: the
[128,2] per-core stats are AllReduced across the 8 cores on-device, BN
scale/shift are computed on-chip (two tiny PE matmuls group and broadcast
the per-channel stats), and y = relu(conv*scale + shift) is applied in
place on the SBUF-resident conv, split across the scalar and vector
engines. The 8 cores split the box along x (host replicates the 1-plane
halo).
"""

import os
import sys

import numpy as np
import ml_dtypes

sys.path.insert(0, "/opt/trn_rl_repo")

N_CORES = 8
C = 16
NBLK = 8
EPS = 1e-5
BF16 = ml_dtypes.bfloat16

OFFS = np.array([(dx, dy, dz)
                 for dx in (-1, 0, 1)
                 for dy in (-1, 0, 1)
                 for dz in (-1, 0, 1)], dtype=np.int64)


# --------------------------------------------------------------------------
# host: grid reconstruction
# --------------------------------------------------------------------------

def _embed(nbr):
    """Recover per-voxel 3D coordinates (up to translation per component)."""
    n = nbr.shape[1]
    pos = np.zeros((n, 3), dtype=np.int32)
    visited = np.zeros(n, dtype=bool)
    comp = np.full(n, -1, dtype=np.int32)
    ks = [k for k in range(27) if k != 13]
    ncomp = 0
    unvis = np.arange(n, dtype=np.int64)
    while unvis.size:
        seed = unvis[0]
        visited[seed] = True
        comp[seed] = ncomp
        frontier = np.array([seed], dtype=np.int64)
        while frontier.size:
            nxt = []
            for k in ks:
                j = nbr[k][frontier]
                m = j < n
                j2 = j[m]
                if j2.size == 0:
                    continue
                newm = ~visited[j2]
                j3 = j2[newm]
                if j3.size == 0:
                    continue
                src = frontier[m][newm]
                uniq, idx = np.unique(j3, return_index=True)
                pos[uniq] = pos[src[idx]] + OFFS[k][None, :].astype(np.int32)
                visited[uniq] = True
                comp[uniq] = ncomp
                nxt.append(uniq)
            frontier = np.concatenate(nxt) if nxt else np.empty(0, np.int64)
        ncomp += 1
        unvis = unvis[~visited[unvis]]

    # per-component bbox, pack along x with 1-plane gaps
    mins = np.full((ncomp, 3), 1 << 29, dtype=np.int64)
    maxs = np.full((ncomp, 3), -(1 << 29), dtype=np.int64)
    for d in range(3):
        np.minimum.at(mins[:, d], comp, pos[:, d])
        np.maximum.at(maxs[:, d], comp, pos[:, d])
    ext = maxs - mins + 1
    sizes = np.bincount(comp, minlength=ncomp)
    order = np.argsort(-sizes)
    # biggest component at the origin; the rest appended along y with a
    # 1-row gap (extending y is cheaper than x: x sets the plane count)
    yoff = np.zeros(ncomp, dtype=np.int64)
    y = 0
    for ci in order:
        yoff[ci] = y
        y += int(ext[ci, 1]) + 1
    out = np.empty((n, 3), dtype=np.int64)
    out[:, 0] = pos[:, 0] - mins[comp, 0]
    out[:, 1] = pos[:, 1] - mins[comp, 1] + yoff[comp]
    out[:, 2] = pos[:, 2] - mins[comp, 2]
    return out, (int(ext[:, 0].max()), y - 1, int(ext[:, 2].max()))


def _verify(nbr, pos, box):
    n = nbr.shape[1]
    bx, by, bz = box
    X2, Y2, Z2 = bx + 2, by + 2, bz + 2
    cell = ((pos[:, 0] + 1) * Y2 + pos[:, 1] + 1) * Z2 + pos[:, 2] + 1
    dense = np.full(X2 * Y2 * Z2, n, dtype=np.int32)
    if np.unique(cell).size != n:
        return False
    dense[cell] = np.arange(n, dtype=np.int32)
    for k in range(27):
        d = (OFFS[k, 0] * Y2 + OFFS[k, 1]) * Z2 + OFFS[k, 2]
        if not np.array_equal(dense[cell + d], nbr[k]):
            return False
    return True


def _prepare_geometry(nbr):
    n = nbr.shape[1]
    pos, (bx, by, bz) = _embed(nbr)
    assert _verify(nbr, pos, (bx, by, bz)), "grid reconstruction failed"
    X2, Y2, Z2 = bx + 2, by + 2, bz + 2
    P = Y2 * Z2
    PB = -(-P // NBLK)                      # cells per block (unpadded)
    NV = -(-PB // 512)                      # tiles per block (<=512 each)
    TF = 2 * (((PB + NV - 1) // NV + 1) >> 1)   # tile width, even, minimal
    BLKP = NV * TF
    assert BLKP >= PB and TF <= 512
    H = Z2 + 1
    COLS = BLKP + 2 * H
    NP = -(-bx // NBLK)                     # output planes per core
    NOUT = NP * NBLK
    xi = pos[:, 0] + 1
    pc = (pos[:, 1] + 1) * Z2 + pos[:, 2] + 1
    return dict(n=n, Z2=Z2, P=P, NV=NV, TF=TF, BLKP=BLKP, H=H, COLS=COLS,
                NP=NP, NOUT=NOUT, xi=xi, pc=pc)


def _build_planes(geo, features):
    P, BLKP, H, COLS, NOUT = (geo[k] for k in
                              ("P", "BLKP", "H", "COLS", "NOUT"))
    xi, pc = geo["xi"], geo["pc"]
    f16 = np.ascontiguousarray(features.astype(BF16))
    G8 = np.zeros((NOUT + 2, 128, COLS), dtype=BF16)
    mask = np.zeros((NOUT, 128, BLKP), dtype=BF16)
    for b in range(NBLK):
        lo, hi = b * BLKP - H, b * BLKP + BLKP + H
        m = (pc >= lo) & (pc < hi)
        G8[xi[m], b * C:(b + 1) * C, pc[m] - lo] = f16[m]
        mb = (pc >= b * BLKP) & (pc < b * BLKP + BLKP)
        mask[xi[mb] - 1, b * C:(b + 1) * C, pc[mb] - b * BLKP] = np.float32(1.0)
    return G8, mask


def _build_wblk(geo, weights):
    Z2 = geo["Z2"]
    wbk = np.zeros((128, 27 * 128), dtype=BF16)
    w16 = weights.astype(BF16)
    for k in range(27):
        for b in range(NBLK):
            wbk[b * C:(b + 1) * C, k * 128 + b * C:k * 128 + (b + 1) * C] = \
                w16[k]
    disp = [(int(OFFS[k, 0]), int(OFFS[k, 1] * Z2 + OFFS[k, 2]))
            for k in range(27)]
    return wbk, disp


# --------------------------------------------------------------------------
# device programs
# --------------------------------------------------------------------------

def _dedup_ldweights(nc):
    """Delete InstLdweights whose weights AP matches the previous load on the
    PE stream (PE array weights persist across matmuls). Runs pre-compile:
    any waits/updates on a deleted load are merged onto the instruction that
    follows it (multi-wait is legal until generate_event_semaphores)."""
    removed = 0
    for f in nc.m.functions:
        for bb in f.blocks:
            insts = bb.instructions
            last_sig = None
            kill = []
            for idx, i in enumerate(insts):
                cn = type(i).__name__
                if cn == "InstLdweights":
                    ap = i.ins[0]
                    sig = (getattr(ap, "offset", None), str(getattr(ap, "ap", "")),
                           str(getattr(i, "perf_mode", None)),
                           str(getattr(i, "is_transpose", None)),
                           str(getattr(i, "tile_position", None)))
                    if sig == last_sig:
                        kill.append(idx)
                    last_sig = sig
                elif cn == "InstMatmult":
                    pass                      # does not disturb loaded weights
                elif getattr(i, "engine", None) is not None and \
                        i.engine == nc.tensor.engine:
                    last_sig = None           # other PE instruction: be safe
            import concourse.mybir as mybir
            for idx in reversed(kill):
                i = insts[idx]
                si = i.sync_info
                if si is not None and (len(si.on_wait) or len(si.on_update)):
                    nxt = insts[idx + 1]
                    nsi = nxt.sync_info
                    ow = list(si.on_wait) + \
                        (list(nsi.on_wait) if nsi is not None else [])
                    ou = (list(nsi.on_update) if nsi is not None else []) + \
                        list(si.on_update)
                    nxt.sync_info = mybir.SyncInfo(on_wait=ow, on_update=ou)
                del insts[idx]
                removed += 1
    return removed


def _build_fused(NP, NV, TF, COLS, H, BLKP, disp):
    """Single NEFF: conv (SBUF-resident) + masked stats + 8-core AllReduce +
    on-device BN constants + relu(conv*scale+shift) + y writeback."""
    import concourse.bacc as bacc
    import concourse.tile as tile
    import concourse.mybir as mybir

    bf16 = mybir.dt.bfloat16
    fp32 = mybir.dt.float32
    NT = NP * NV

    nc = bacc.Bacc("TRN2", target_bir_lowering=False, debug=False,
                   num_devices=N_CORES)
    g8 = nc.dram_tensor("g8", [NP + 2, 128, COLS], bf16, kind="ExternalInput")
    msk = nc.dram_tensor("msk", [NP, 128, BLKP], bf16, kind="ExternalInput")
    wbk = nc.dram_tensor("wbk", [128, 27 * 128], bf16, kind="ExternalInput")
    gm = nc.dram_tensor("gm", [128, 16], fp32, kind="ExternalInput")
    gt = nc.dram_tensor("gt", [16, 128], fp32, kind="ExternalInput")
    gb = nc.dram_tensor("gb", [16, 2], fp32, kind="ExternalInput")
    y_d = nc.dram_tensor("y", [NP, 128, BLKP], bf16, kind="ExternalOutput")
    cc_in = nc.dram_tensor("cc_in", [128, 2], fp32)
    cc_out = nc.dram_tensor("cc_out", [128, 2], fp32)
    cc_in2 = nc.dram_tensor("cc_in2", [128, 2], fp32)
    cc_out2 = nc.dram_tensor("cc_out2", [128, 2], fp32)

    with tile.TileContext(nc) as tc:
        with (
            tc.tile_pool(name="res", bufs=1) as res_pool,
            tc.tile_pool(name="gp", bufs=5) as gp,
            tc.tile_pool(name="mp", bufs=3) as mp,
            tc.tile_pool(name="cm", bufs=3) as cmp_,
            tc.tile_pool(name="sq", bufs=2) as sqp,
            tc.tile_pool(name="ps", bufs=4, space="PSUM") as psp,
            tc.tile_pool(name="pe", bufs=1, space="PSUM") as pep,
        ):
            w_sb = res_pool.tile([128, 27 * 128], bf16)
            nc.sync.dma_start(w_sb[:], wbk[:])
            sacc_s = res_pool.tile([128, NT], fp32)
            sacc_q = res_pool.tile([128, NT], fp32)
            conv_sb = res_pool.tile([128, NP * BLKP], bf16)
            gm_sb = res_pool.tile([128, 16], fp32)
            gt_sb = res_pool.tile([16, 128], fp32)
            gb_sb = res_pool.tile([16, 2], fp32)
            nc.sync.dma_start(gm_sb[:], gm[:])
            nc.sync.dma_start(gt_sb[:], gt[:])
            nc.sync.dma_start(gb_sb[:], gb[:])

            g_tiles = {}

            def load_g(i):
                t = gp.tile([128, COLS], bf16, tag="g")
                nc.sync.dma_start(t[:], g8[i])
                g_tiles[i] = t

            for i in range(min(3, NP + 2)):
                load_g(i)

            GSZ = 1                     # PSUM tiles per k-sweep group
            for o in range(NP):
                if o + 3 < NP + 2:
                    load_g(o + 3)
                m_t = mp.tile([128, BLKP], bf16, tag="m")
                nc.sync.dma_start(m_t[:], msk[o])
                for j0 in range(0, NV, GSZ):
                    js = list(range(j0, min(j0 + GSZ, NV)))
                    ps_ts = []
                    for j in js:
                        ps_t = psp.tile([128, TF], fp32, tag=f"p{j - j0}")
                        ps_ts.append(ps_t)
                    # k-outer: one weight load serves the whole group (the
                    # redundant per-matmul reloads are removed pre-compile)
                    for k in range(27):
                        dx, dc = disp[k]
                        g_in = g_tiles[o + 1 + dx]
                        for j, ps_t in zip(js, ps_ts):
                            c0 = H + j * TF + dc
                            nc.tensor.matmul(
                                ps_t[:],
                                lhsT=w_sb[:, k * 128:(k + 1) * 128],
                                rhs=g_in[:, c0:c0 + TF],
                                start=(k == 0),
                                stop=(k == 26),
                            )
                    for j, ps_t in zip(js, ps_ts):
                        t = o * NV + j
                        cm_t = cmp_.tile([128, TF], fp32, tag="cm")
                        nc.vector.tensor_mul(
                            out=cm_t[:], in0=ps_t[:],
                            in1=m_t[:, j * TF:(j + 1) * TF])
                        nc.scalar.activation(
                            out=conv_sb[:, o * BLKP + j * TF:
                                        o * BLKP + (j + 1) * TF],
                            in_=cm_t[:],
                            func=mybir.ActivationFunctionType.Copy,
                            accum_out=sacc_s[:, t:t + 1])
                        sq_t = sqp.tile([128, TF], fp32, tag="sq")
                        nc.scalar.activation(
                            out=sq_t[:], in_=cm_t[:],
                            func=mybir.ActivationFunctionType.Square,
                            accum_out=sacc_q[:, t:t + 1])
                if o == NP - 2:
                    # partial stats (planes [0, NP-1)): AllReduce overlaps
                    # the last plane's matmuls
                    stA = res_pool.tile([128, 2], fp32)
                    TA = (NP - 1) * NV
                    nc.vector.tensor_reduce(out=stA[:, 0:1],
                                            in_=sacc_s[:, :TA],
                                            axis=mybir.AxisListType.X,
                                            op=mybir.AluOpType.add)
                    nc.vector.tensor_reduce(out=stA[:, 1:2],
                                            in_=sacc_q[:, :TA],
                                            axis=mybir.AxisListType.X,
                                            op=mybir.AluOpType.add)
                    nc.sync.dma_start(cc_in[:], stA[:])
                    nc.gpsimd.collective_compute(
                        "AllReduce", mybir.AluOpType.add,
                        replica_groups=[list(range(N_CORES))],
                        ins=[cc_in[:].opt()], outs=[cc_out[:].opt()])
                if o == NP - 1:
                    stB = res_pool.tile([128, 2], fp32)
                    TA = (NP - 1) * NV
                    nc.vector.tensor_reduce(out=stB[:, 0:1],
                                            in_=sacc_s[:, TA:],
                                            axis=mybir.AxisListType.X,
                                            op=mybir.AluOpType.add)
                    nc.vector.tensor_reduce(out=stB[:, 1:2],
                                            in_=sacc_q[:, TA:],
                                            axis=mybir.AxisListType.X,
                                            op=mybir.AluOpType.add)
                    nc.sync.dma_start(cc_in2[:], stB[:])
                    nc.gpsimd.collective_compute(
                        "AllReduce", mybir.AluOpType.add,
                        replica_groups=[list(range(N_CORES))],
                        ins=[cc_in2[:].opt()], outs=[cc_out2[:].opt()])

            st_r = res_pool.tile([128, 4], fp32)
            nc.sync.dma_start(st_r[:, 0:2], cc_out[:])
            nc.sync.dma_start(st_r[:, 2:4], cc_out2[:])
            nc.vector.tensor_add(out=st_r[:, 0:2], in0=st_r[:, 0:2],
                                 in1=st_r[:, 2:4])

            # per-channel mean/E[x^2]: group the 8 block rows (gm = delta/n)
            ps16 = pep.tile([16, 2], fp32, tag="st16")
            nc.tensor.matmul(ps16[:], lhsT=gm_sb[:], rhs=st_r[:, 0:2],
                             start=True, stop=True)
            m16 = res_pool.tile([16, 2], fp32)
            nc.vector.tensor_copy(out=m16[:], in_=ps16[:])
            v16 = res_pool.tile([16, 3], fp32)
            nc.vector.tensor_mul(out=v16[:, 0:1], in0=m16[:, 0:1],
                                 in1=m16[:, 0:1])
            nc.vector.tensor_tensor(out=v16[:, 1:2], in0=m16[:, 1:2],
                                    in1=v16[:, 0:1],
                                    op=mybir.AluOpType.subtract)
            nc.vector.tensor_scalar_add(out=v16[:, 1:2], in0=v16[:, 1:2],
                                        scalar1=float(EPS))
            nc.scalar.activation(out=v16[:, 2:3], in_=v16[:, 1:2],
                                 func=mybir.ActivationFunctionType.Sqrt)
            sc16 = res_pool.tile([16, 2], fp32)
            inv16 = res_pool.tile([16, 1], fp32)
            nc.vector.reciprocal(out=inv16[:], in_=v16[:, 2:3])
            nc.vector.tensor_mul(out=sc16[:, 0:1], in0=gb_sb[:, 0:1],
                                 in1=inv16[:])
            tmp16 = res_pool.tile([16, 1], fp32)
            nc.vector.tensor_mul(out=tmp16[:], in0=m16[:, 0:1],
                                 in1=sc16[:, 0:1])
            nc.vector.tensor_tensor(out=sc16[:, 1:2], in0=gb_sb[:, 1:2],
                                    in1=tmp16[:],
                                    op=mybir.AluOpType.subtract)
            # broadcast [16,2] -> [128,2] (gt = delta^T)
            psb = pep.tile([128, 2], fp32, tag="bc")
            nc.tensor.matmul(psb[:], lhsT=gt_sb[:], rhs=sc16[:],
                             start=True, stop=True)
            sc_sb = res_pool.tile([128, 2], fp32)
            nc.vector.tensor_copy(out=sc_sb[:], in_=psb[:])

            # y = relu(conv*scale + shift), in place, ACT/DVE/GPSIMD split
            c1 = 2 * (BLKP * 2 // 5 // 2)
            c2 = c1 + 2 * (BLKP * 19 // 50 // 2)
            for o in range(NP):
                sl_a = conv_sb[:, o * BLKP:o * BLKP + c1]
                sl_b = conv_sb[:, o * BLKP + c1:o * BLKP + c2]
                sl_c = conv_sb[:, o * BLKP + c2:(o + 1) * BLKP]
                nc.scalar.activation(
                    out=sl_a, in_=sl_a,
                    func=mybir.ActivationFunctionType.Relu,
                    bias=sc_sb[:, 1:2], scale=sc_sb[:, 0:1])
                nc.sync.dma_start(y_d[o][:, :c1],
                                  conv_sb[:, o * BLKP:o * BLKP + c1])
                nc.vector.tensor_scalar(
                    out=sl_b, in0=sl_b,
                    scalar1=sc_sb[:, 0:1], scalar2=sc_sb[:, 1:2],
                    op0=mybir.AluOpType.mult, op1=mybir.AluOpType.add)
                nc.vector.tensor_scalar_max(out=sl_b, in0=sl_b, scalar1=0.0)
                nc.sync.dma_start(y_d[o][:, c1:c2],
                                  conv_sb[:, o * BLKP + c1:o * BLKP + c2])
                nc.gpsimd.tensor_scalar(
                    out=sl_c, in0=sl_c,
                    scalar1=sc_sb[:, 0:1], scalar2=sc_sb[:, 1:2],
                    op0=mybir.AluOpType.mult, op1=mybir.AluOpType.add)
                nc.gpsimd.tensor_scalar_max(out=sl_c, in0=sl_c, scalar1=0.0)
                nc.sync.dma_start(y_d[o][:, c2:],
                                  conv_sb[:, o * BLKP + c2:(o + 1) * BLKP])

    _dedup_ldweights(nc)
    nc.compile()
    return nc


def _build_pass2(NP, NV, TF, BLKP):
    import concourse.bacc as bacc
    import concourse.tile as tile
    import concourse.mybir as mybir

    bf16 = mybir.dt.bfloat16
    fp32 = mybir.dt.float32

    nc = bacc.Bacc("TRN2", target_bir_lowering=False, debug=False,
                   num_devices=N_CORES)
    conv_d = nc.dram_tensor("conv", [NP, 128, BLKP], bf16,
                            kind="ExternalInput")
    sc = nc.dram_tensor("sc", [128, 2], fp32, kind="ExternalInput")
    y_d = nc.dram_tensor("y", [NP, 128, BLKP], bf16, kind="ExternalOutput")

    with tile.TileContext(nc) as tc:
        with (
            tc.tile_pool(name="res", bufs=1) as res_pool,
            tc.tile_pool(name="yin", bufs=4) as yip,
            tc.tile_pool(name="yout", bufs=4) as yop,
        ):
            sc_sb = res_pool.tile([128, 2], fp32)
            nc.sync.dma_start(sc_sb[:], sc[:])
            half = NV * TF // 2
            for o in range(NP):
                ci = yip.tile([128, NV * TF], bf16, tag="ci")
                nc.sync.dma_start(ci[:], conv_d[o])
                yo = yop.tile([128, NV * TF], bf16, tag="yo")
                # split each plane between the scalar and vector engines
                nc.scalar.activation(
                    out=yo[:, :half], in_=ci[:, :half],
                    func=mybir.ActivationFunctionType.Relu,
                    bias=sc_sb[:, 1:2], scale=sc_sb[:, 0:1],
                )
                nc.vector.tensor_scalar(
                    out=yo[:, half:], in0=ci[:, half:],
                    scalar1=sc_sb[:, 0:1], scalar2=sc_sb[:, 1:2],
                    op0=mybir.AluOpType.mult, op1=mybir.AluOpType.add,
                )
                nc.vector.tensor_scalar_max(out=yo[:, half:], in0=yo[:, half:],
                                            scalar1=0.0)
                nc.sync.dma_start(y_d[o], yo[:])
    nc.compile()
    return nc


# --------------------------------------------------------------------------
# tracing plumbing: make sure the NTFF profile hook exists
# --------------------------------------------------------------------------

def _ensure_trace_hook():
    try:
        import antenv
        try:
            from antenv.axon_hooks import get_axon_ntff_profile_hook
            if get_axon_ntff_profile_hook() is not None:
                return True
        except ImportError:
            import types
            mod = types.ModuleType("antenv.axon_hooks")
            mod._hook = None

            def set_axon_ntff_profile_hook(h):
                mod._hook = h

            def get_axon_ntff_profile_hook():
                return mod._hook

            mod.set_axon_ntff_profile_hook = set_axon_ntff_profile_hook
            mod.get_axon_ntff_profile_hook = get_axon_ntff_profile_hook
            sys.modules["antenv.axon_hooks"] = mod
            antenv.axon_hooks = mod

        # register the ctypes-based hook if libaxon is present
        import contextlib
        import ctypes
        from antenv.axon_hooks import (get_axon_ntff_profile_hook,
                                       set_axon_ntff_profile_hook)
        so_path = "/opt/axon/libaxon_pjrt.so"
        if not os.path.exists(so_path):
            return False
        lib = ctypes.CDLL(so_path)
        if not hasattr(lib, "axon_start_nrt_profile"):
            return False
        lib.axon_start_nrt_profile.argtypes = [
            ctypes.POINTER(ctypes.c_int64), ctypes.c_size_t]
        lib.axon_start_nrt_profile.restype = ctypes.c_int64
        lib.axon_stop_nrt_profile.argtypes = [ctypes.c_char_p]
        lib.axon_stop_nrt_profile.restype = ctypes.c_int64

        @contextlib.contextmanager
        def _hook(output_dir, device_ids):
            import jax
            jax.devices()
            if device_ids:
                ids = (ctypes.c_int64 * len(device_ids))(*device_ids)
                rc = lib.axon_start_nrt_profile(ids, len(device_ids))
            else:
                rc = lib.axon_start_nrt_profile(None, 0)
            if rc != 0:
                raise RuntimeError(f"axon_start_nrt_profile rc={rc}")
            try:
                yield
            finally:
                nf = lib.axon_stop_nrt_profile(str(output_dir).encode())
                if nf < 0:
                    raise RuntimeError(f"axon_stop_nrt_profile rc={nf}")

        set_axon_ntff_profile_hook(_hook)
        return True
    except Exception:
        return False


# --------------------------------------------------------------------------
# entry point
# --------------------------------------------------------------------------

_CACHE = {}


def kernel(features, weights, gamma, beta, neighbor_idx):
    from concourse.bass_utils import run_bass_kernel_spmd

    features = np.asarray(features, dtype=np.float32)
    weights = np.asarray(weights, dtype=np.float32)
    gamma = np.asarray(gamma, dtype=np.float32)
    beta = np.asarray(beta, dtype=np.float32)
    nbr = np.asarray(neighbor_idx, dtype=np.int32)
    n = features.shape[0]

    trace = os.environ.get("KERNEL_TRACE", "1") == "1"
    if trace:
        trace = _ensure_trace_hook()

    geo = _prepare_geometry(nbr)
    G8, mask = _build_planes(geo, features)
    wbk, disp = _build_wblk(geo, weights)
    NP, NV, TFg, COLS, H, BLKP, NOUT = (geo[k] for k in
                                        ("NP", "NV", "TF", "COLS", "H",
                                         "BLKP", "NOUT"))

    key = (NP, NV, TFg, COLS)
    if key not in _CACHE:
        _CACHE[key] = _build_fused(NP, NV, TFg, COLS, H, BLKP, disp)
    nc1 = _CACHE[key]

    total_ns = 0

    # grouping matrices for the on-device BN constant computation
    gmh = np.zeros((128, 16), dtype=np.float32)
    gth = np.zeros((16, 128), dtype=np.float32)
    for p in range(128):
        gmh[p, p % C] = 1.0 / n
        gth[p % C, p] = 1.0
    gbh = np.stack([gamma, beta], axis=1).astype(np.float32)

    in_maps = [{"g8": G8[c * NP:c * NP + NP + 2],
                "msk": mask[c * NP:(c + 1) * NP],
                "wbk": wbk, "gm": gmh, "gt": gth, "gb": gbh}
               for c in range(N_CORES)]
    res1 = run_bass_kernel_spmd(nc1, in_maps, core_ids=list(range(N_CORES)),
                                trace=trace)
    if res1.exec_time_ns is not None:
        total_ns += res1.exec_time_ns

    if total_ns:
        print(f"HW exec time: {total_ns} ns")

    y = np.concatenate([res1.results[c]["y"] for c in range(N_CORES)], axis=0)

    xi, pc = geo["xi"], geo["pc"]
    b = pc // BLKP
    col = pc - b * BLKP
    out = np.empty((n, C), dtype=np.float32)
    for ch in range(C):
        out[:, ch] = y[xi - 1, b * C + ch, col].astype(np.float32)
    return out


# revision 22
# speedup vs baseline: 1.2045x; 1.2045x over previous
"""Trainium2 kernel for nn_Basic3DBlock (sparse 3D conv + sync BN + ReLU).

Strategy: the neighbor map was generated from a hidden embedding of the N
voxels into a dense 3D grid (27-point stencil). The host reconstructs exact
relative coordinates from neighbor_idx by BFS over the 26 directed neighbor
relations (components packed into a zero-padded box), turning the sparse
gather-conv into a DENSE 27-tap stencil — no indirect DMA at all.

Device layout: one x-plane of the box is stored channel-major in 8 blocks
stacked on the partition axis with an in-plane halo: tile [128, COLS],
partition (b*16+c) col v = channel c of plane cell (b*BLKP + v - H). All 9
in-plane shifts are column offsets of that tile; the x-shifts pick one of 3
resident plane tiles. Weights become 27 block-diagonal [128,128] bf16
matrices; 27 accumulating matmuls per [128,512] PSUM tile compute conv for
4096 cells. A mask (1 at occupied cells) zeroes garbage at empty cells and
feeds masked sum/sumsq row-reductions for sync BN (free on the scalar
engine's activation accumulator). Everything runs in ONE NEFF launch: the
[128,2] per-core stats are AllReduced across the 8 cores on-device, BN
scale/shift are computed on-chip (two tiny PE matmuls group and broadcast
the per-channel stats), and y = relu(conv*scale + shift) is applied in
place on the SBUF-resident conv, split across the scalar and vector
engines. The 8 cores split the box along x (host replicates the 1-plane
halo).
"""

import os
import sys

import numpy as np
import ml_dtypes

sys.path.insert(0, "/opt/trn_rl_repo")

N_CORES = 8
C = 16
NBLK = 8
EPS = 1e-5
BF16 = ml_dtypes.bfloat16

OFFS = np.array([(dx, dy, dz)
                 for dx in (-1, 0, 1)
                 for dy in (-1, 0, 1)
                 for dz in (-1, 0, 1)], dtype=np.int64)


# --------------------------------------------------------------------------
# host: grid reconstruction
# --------------------------------------------------------------------------

def _embed(nbr):
    """Recover per-voxel 3D coordinates (up to translation per component)."""
    n = nbr.shape[1]
    pos = np.zeros((n, 3), dtype=np.int32)
    visited = np.zeros(n, dtype=bool)
    comp = np.full(n, -1, dtype=np.int32)
    ks = [k for k in range(27) if k != 13]
    ncomp = 0
    unvis = np.arange(n, dtype=np.int64)
    while unvis.size:
        seed = unvis[0]
        visited[seed] = True
        comp[seed] = ncomp
        frontier = np.array([seed], dtype=np.int64)
        while frontier.size:
            nxt = []
            for k in ks:
                j = nbr[k][frontier]
                m = j < n
                j2 = j[m]
                if j2.size == 0:
                    continue
                newm = ~visited[j2]
                j3 = j2[newm]
                if j3.size == 0:
                    continue
                src = frontier[m][newm]
                uniq, idx = np.unique(j3, return_index=True)
                pos[uniq] = pos[src[idx]] + OFFS[k][None, :].astype(np.int32)
                visited[uniq] = True
                comp[uniq] = ncomp
                nxt.append(uniq)
            frontier = np.concatenate(nxt) if nxt else np.empty(0, np.int64)
        ncomp += 1
        unvis = unvis[~visited[unvis]]

    # per-component bbox, pack along x with 1-plane gaps
    mins = np.full((ncomp, 3), 1 << 29, dtype=np.int64)
    maxs = np.full((ncomp, 3), -(1 << 29), dtype=np.int64)
    for d in range(3):
        np.minimum.at(mins[:, d], comp, pos[:, d])
        np.maximum.at(maxs[:, d], comp, pos[:, d])
    ext = maxs - mins + 1
    sizes = np.bincount(comp, minlength=ncomp)
    order = np.argsort(-sizes)
    # biggest component at the origin; the rest appended along y with a
    # 1-row gap (extending y is cheaper than x: x sets the plane count)
    yoff = np.zeros(ncomp, dtype=np.int64)
    y = 0
    for ci in order:
        yoff[ci] = y
        y += int(ext[ci, 1]) + 1
    out = np.empty((n, 3), dtype=np.int64)
    out[:, 0] = pos[:, 0] - mins[comp, 0]
    out[:, 1] = pos[:, 1] - mins[comp, 1] + yoff[comp]
    out[:, 2] = pos[:, 2] - mins[comp, 2]
    return out, (int(ext[:, 0].max()), y - 1, int(ext[:, 2].max()))


def _verify(nbr, pos, box):
    n = nbr.shape[1]
    bx, by, bz = box
    X2, Y2, Z2 = bx + 2, by + 2, bz + 2
    cell = ((pos[:, 0] + 1) * Y2 + pos[:, 1] + 1) * Z2 + pos[:, 2] + 1
    dense = np.full(X2 * Y2 * Z2, n, dtype=np.int32)
    if np.unique(cell).size != n:
        return False
    dense[cell] = np.arange(n, dtype=np.int32)
    for k in range(27):
        d = (OFFS[k, 0] * Y2 + OFFS[k, 1]) * Z2 + OFFS[k, 2]
        if not np.array_equal(dense[cell + d], nbr[k]):
            return False
    return True


def _prepare_geometry(nbr):
    n = nbr.shape[1]
    pos, (bx, by, bz) = _embed(nbr)
    assert _verify(nbr, pos, (bx, by, bz)), "grid reconstruction failed"
    X2, Y2, Z2 = bx + 2, by + 2, bz + 2
    P = Y2 * Z2
    PB = -(-P // NBLK)                      # cells per block (unpadded)
    NV = -(-PB // 512)                      # tiles per block (<=512 each)
    TF = 2 * (((PB + NV - 1) // NV + 1) >> 1)   # tile width, even, minimal
    BLKP = NV * TF
    assert BLKP >= PB and TF <= 512
    H = Z2 + 1
    COLS = BLKP + 2 * H
    NP = -(-bx // NBLK)                     # output planes per core
    NOUT = NP * NBLK
    xi = pos[:, 0] + 1
    pc = (pos[:, 1] + 1) * Z2 + pos[:, 2] + 1
    return dict(n=n, Z2=Z2, P=P, NV=NV, TF=TF, BLKP=BLKP, H=H, COLS=COLS,
                NP=NP, NOUT=NOUT, xi=xi, pc=pc)


def _build_planes(geo, features):
    P, BLKP, H, COLS, NOUT = (geo[k] for k in
                              ("P", "BLKP", "H", "COLS", "NOUT"))
    xi, pc = geo["xi"], geo["pc"]
    f16 = np.ascontiguousarray(features.astype(BF16))
    G8 = np.zeros((NOUT + 2, 128, COLS), dtype=BF16)
    mask = np.zeros((NOUT, 128, BLKP), dtype=BF16)
    for b in range(NBLK):
        lo, hi = b * BLKP - H, b * BLKP + BLKP + H
        m = (pc >= lo) & (pc < hi)
        G8[xi[m], b * C:(b + 1) * C, pc[m] - lo] = f16[m]
        mb = (pc >= b * BLKP) & (pc < b * BLKP + BLKP)
        mask[xi[mb] - 1, b * C:(b + 1) * C, pc[mb] - b * BLKP] = np.float32(1.0)
    return G8, mask


def _build_wblk(geo, weights):
    Z2 = geo["Z2"]
    wbk = np.zeros((128, 27 * 128), dtype=BF16)
    w16 = weights.astype(BF16)
    for k in range(27):
        for b in range(NBLK):
            wbk[b * C:(b + 1) * C, k * 128 + b * C:k * 128 + (b + 1) * C] = \
                w16[k]
    disp = [(int(OFFS[k, 0]), int(OFFS[k, 1] * Z2 + OFFS[k, 2]))
            for k in range(27)]
    return wbk, disp


# --------------------------------------------------------------------------
# device programs
# --------------------------------------------------------------------------

def _dedup_ldweights(nc):
    """Delete InstLdweights whose weights AP matches the previous load on the
    PE stream (PE array weights persist across matmuls). Runs pre-compile:
    any waits/updates on a deleted load are merged onto the instruction that
    follows it (multi-wait is legal until generate_event_semaphores)."""
    removed = 0
    for f in nc.m.functions:
        for bb in f.blocks:
            insts = bb.instructions
            last_sig = None
            kill = []
            for idx, i in enumerate(insts):
                cn = type(i).__name__
                if cn == "InstLdweights":
                    ap = i.ins[0]
                    sig = (getattr(ap, "offset", None), str(getattr(ap, "ap", "")),
                           str(getattr(i, "perf_mode", None)),
                           str(getattr(i, "is_transpose", None)),
                           str(getattr(i, "tile_position", None)))
                    if sig == last_sig:
                        kill.append(idx)
                    last_sig = sig
                elif cn == "InstMatmult":
                    pass                      # does not disturb loaded weights
                elif getattr(i, "engine", None) is not None and \
                        i.engine == nc.tensor.engine:
                    last_sig = None           # other PE instruction: be safe
            import concourse.mybir as mybir
            for idx in reversed(kill):
                i = insts[idx]
                si = i.sync_info
                if si is not None and (len(si.on_wait) or len(si.on_update)):
                    nxt = insts[idx + 1]
                    nsi = nxt.sync_info
                    ow = list(si.on_wait) + \
                        (list(nsi.on_wait) if nsi is not None else [])
                    ou = (list(nsi.on_update) if nsi is not None else []) + \
                        list(si.on_update)
                    nxt.sync_info = mybir.SyncInfo(on_wait=ow, on_update=ou)
                del insts[idx]
                removed += 1
    return removed


def _build_fused(NP, NV, TF, COLS, H, BLKP, disp):
    """Single NEFF: conv (SBUF-resident) + masked stats + 8-core AllReduce +
    on-device BN constants + relu(conv*scale+shift) + y writeback."""
    import concourse.bacc as bacc
    import concourse.tile as tile
    import concourse.mybir as mybir

    bf16 = mybir.dt.bfloat16
    fp32 = mybir.dt.float32
    NT = NP * NV

    nc = bacc.Bacc("TRN2", target_bir_lowering=False, debug=False,
                   num_devices=N_CORES)
    g8 = nc.dram_tensor("g8", [NP + 2, 128, COLS], bf16, kind="ExternalInput")
    msk = nc.dram_tensor("msk", [NP, 128, BLKP], bf16, kind="ExternalInput")
    wbk = nc.dram_tensor("wbk", [128, 27 * 128], bf16, kind="ExternalInput")
    gm = nc.dram_tensor("gm", [128, 16], fp32, kind="ExternalInput")
    gt = nc.dram_tensor("gt", [16, 128], fp32, kind="ExternalInput")
    gb = nc.dram_tensor("gb", [16, 2], fp32, kind="ExternalInput")
    y_d = nc.dram_tensor("y", [NP, 128, BLKP], bf16, kind="ExternalOutput")
    cc_in = nc.dram_tensor("cc_in", [128, 2], fp32)
    cc_out = nc.dram_tensor("cc_out", [128, 2], fp32)

    with tile.TileContext(nc) as tc:
        with (
            tc.tile_pool(name="res", bufs=1) as res_pool,
            tc.tile_pool(name="gp", bufs=5) as gp,
            tc.tile_pool(name="mp", bufs=3) as mp,
            tc.tile_pool(name="cm", bufs=3) as cmp_,
            tc.tile_pool(name="sq", bufs=2) as sqp,
            tc.tile_pool(name="ps", bufs=4, space="PSUM") as psp,
            tc.tile_pool(name="pe", bufs=1, space="PSUM") as pep,
        ):
            w_sb = res_pool.tile([128, 27 * 128], bf16)
            nc.sync.dma_start(w_sb[:], wbk[:])
            sacc_s = res_pool.tile([128, NT], fp32)
            sacc_q = res_pool.tile([128, NT], fp32)
            conv_sb = res_pool.tile([128, NP * BLKP], bf16)
            gm_sb = res_pool.tile([128, 16], fp32)
            gt_sb = res_pool.tile([16, 128], fp32)
            gb_sb = res_pool.tile([16, 2], fp32)
            nc.sync.dma_start(gm_sb[:], gm[:])
            nc.sync.dma_start(gt_sb[:], gt[:])
            nc.sync.dma_start(gb_sb[:], gb[:])

            g_tiles = {}

            def load_g(i):
                t = gp.tile([128, COLS], bf16, tag="g")
                nc.sync.dma_start(t[:], g8[i])
                g_tiles[i] = t

            for i in range(min(3, NP + 2)):
                load_g(i)

            GSZ = 1                     # PSUM tiles per k-sweep group
            for o in range(NP):
                if o + 3 < NP + 2:
                    load_g(o + 3)
                m_t = mp.tile([128, BLKP], bf16, tag="m")
                nc.sync.dma_start(m_t[:], msk[o])
                for j0 in range(0, NV, GSZ):
                    js = list(range(j0, min(j0 + GSZ, NV)))
                    ps_ts = []
                    for j in js:
                        ps_t = psp.tile([128, TF], fp32, tag=f"p{j - j0}")
                        ps_ts.append(ps_t)
                    # k-outer: one weight load serves the whole group (the
                    # redundant per-matmul reloads are removed pre-compile)
                    for k in range(27):
                        dx, dc = disp[k]
                        g_in = g_tiles[o + 1 + dx]
                        for j, ps_t in zip(js, ps_ts):
                            c0 = H + j * TF + dc
                            nc.tensor.matmul(
                                ps_t[:],
                                lhsT=w_sb[:, k * 128:(k + 1) * 128],
                                rhs=g_in[:, c0:c0 + TF],
                                start=(k == 0),
                                stop=(k == 26),
                            )
                    for j, ps_t in zip(js, ps_ts):
                        t = o * NV + j
                        cm_t = cmp_.tile([128, TF], fp32, tag="cm")
                        nc.vector.tensor_mul(
                            out=cm_t[:], in0=ps_t[:],
                            in1=m_t[:, j * TF:(j + 1) * TF])
                        nc.scalar.activation(
                            out=conv_sb[:, o * BLKP + j * TF:
                                        o * BLKP + (j + 1) * TF],
                            in_=cm_t[:],
                            func=mybir.ActivationFunctionType.Copy,
                            accum_out=sacc_s[:, t:t + 1])
                        sq_t = sqp.tile([128, TF], fp32, tag="sq")
                        nc.scalar.activation(
                            out=sq_t[:], in_=cm_t[:],
                            func=mybir.ActivationFunctionType.Square,
                            accum_out=sacc_q[:, t:t + 1])

            st = res_pool.tile([128, 2], fp32)
            nc.vector.tensor_reduce(out=st[:, 0:1], in_=sacc_s[:],
                                    axis=mybir.AxisListType.X,
                                    op=mybir.AluOpType.add)
            nc.vector.tensor_reduce(out=st[:, 1:2], in_=sacc_q[:],
                                    axis=mybir.AxisListType.X,
                                    op=mybir.AluOpType.add)

            # sync BN: AllReduce the [128,2] stats across the 8 cores
            nc.sync.dma_start(cc_in[:], st[:])
            nc.gpsimd.collective_compute(
                "AllReduce", mybir.AluOpType.add,
                replica_groups=[list(range(N_CORES))],
                ins=[cc_in[:].opt()], outs=[cc_out[:].opt()])
            st_r = res_pool.tile([128, 2], fp32)
            nc.sync.dma_start(st_r[:], cc_out[:])

            # per-channel mean/E[x^2]: group the 8 block rows (gm = delta/n)
            ps16 = pep.tile([16, 2], fp32, tag="st16")
            nc.tensor.matmul(ps16[:], lhsT=gm_sb[:], rhs=st_r[:],
                             start=True, stop=True)
            m16 = res_pool.tile([16, 2], fp32)
            nc.vector.tensor_copy(out=m16[:], in_=ps16[:])
            v16 = res_pool.tile([16, 3], fp32)
            nc.vector.tensor_mul(out=v16[:, 0:1], in0=m16[:, 0:1],
                                 in1=m16[:, 0:1])
            nc.vector.tensor_tensor(out=v16[:, 1:2], in0=m16[:, 1:2],
                                    in1=v16[:, 0:1],
                                    op=mybir.AluOpType.subtract)
            nc.vector.tensor_scalar_add(out=v16[:, 1:2], in0=v16[:, 1:2],
                                        scalar1=float(EPS))
            nc.scalar.activation(out=v16[:, 2:3], in_=v16[:, 1:2],
                                 func=mybir.ActivationFunctionType.Sqrt)
            sc16 = res_pool.tile([16, 2], fp32)
            inv16 = res_pool.tile([16, 1], fp32)
            nc.vector.reciprocal(out=inv16[:], in_=v16[:, 2:3])
            nc.vector.tensor_mul(out=sc16[:, 0:1], in0=gb_sb[:, 0:1],
                                 in1=inv16[:])
            tmp16 = res_pool.tile([16, 1], fp32)
            nc.vector.tensor_mul(out=tmp16[:], in0=m16[:, 0:1],
                                 in1=sc16[:, 0:1])
            nc.vector.tensor_tensor(out=sc16[:, 1:2], in0=gb_sb[:, 1:2],
                                    in1=tmp16[:],
                                    op=mybir.AluOpType.subtract)
            # broadcast [16,2] -> [128,2] (gt = delta^T)
            psb = pep.tile([128, 2], fp32, tag="bc")
            nc.tensor.matmul(psb[:], lhsT=gt_sb[:], rhs=sc16[:],
                             start=True, stop=True)
            sc_sb = res_pool.tile([128, 2], fp32)
            nc.vector.tensor_copy(out=sc_sb[:], in_=psb[:])

            # y = relu(conv*scale + shift), in place, split ACT/DVE 40/60
            half = 2 * (BLKP * 2 // 5 // 2)
            for o in range(NP):
                sl_a = conv_sb[:, o * BLKP:o * BLKP + half]
                sl_b = conv_sb[:, o * BLKP + half:(o + 1) * BLKP]
                nc.scalar.activation(
                    out=sl_a, in_=sl_a,
                    func=mybir.ActivationFunctionType.Relu,
                    bias=sc_sb[:, 1:2], scale=sc_sb[:, 0:1])
                nc.sync.dma_start(y_d[o][:, :half],
                                  conv_sb[:, o * BLKP:o * BLKP + half])
                nc.vector.tensor_scalar(
                    out=sl_b, in0=sl_b,
                    scalar1=sc_sb[:, 0:1], scalar2=sc_sb[:, 1:2],
                    op0=mybir.AluOpType.mult, op1=mybir.AluOpType.add)
                nc.vector.tensor_scalar_max(out=sl_b, in0=sl_b, scalar1=0.0)
                nc.sync.dma_start(y_d[o][:, half:],
                                  conv_sb[:, o * BLKP + half:(o + 1) * BLKP])

    _dedup_ldweights(nc)
    nc.compile()
    return nc


def _build_pass2(NP, NV, TF, BLKP):
    import concourse.bacc as bacc
    import concourse.tile as tile
    import concourse.mybir as mybir

    bf16 = mybir.dt.bfloat16
    fp32 = mybir.dt.float32

    nc = bacc.Bacc("TRN2", target_bir_lowering=False, debug=False,
                   num_devices=N_CORES)
    conv_d = nc.dram_tensor("conv", [NP, 128, BLKP], bf16,
                            kind="ExternalInput")
    sc = nc.dram_tensor("sc", [128, 2], fp32, kind="ExternalInput")
    y_d = nc.dram_tensor("y", [NP, 128, BLKP], bf16, kind="ExternalOutput")

    with tile.TileContext(nc) as tc:
        with (
            tc.tile_pool(name="res", bufs=1) as res_pool,
            tc.tile_pool(name="yin", bufs=4) as yip,
            tc.tile_pool(name="yout", bufs=4) as yop,
        ):
            sc_sb = res_pool.tile([128, 2], fp32)
            nc.sync.dma_start(sc_sb[:], sc[:])
            half = NV * TF // 2
            for o in range(NP):
                ci = yip.tile([128, NV * TF], bf16, tag="ci")
                nc.sync.dma_start(ci[:], conv_d[o])
                yo = yop.tile([128, NV * TF], bf16, tag="yo")
                # split each plane between the scalar and vector engines
                nc.scalar.activation(
                    out=yo[:, :half], in_=ci[:, :half],
                    func=mybir.ActivationFunctionType.Relu,
                    bias=sc_sb[:, 1:2], scale=sc_sb[:, 0:1],
                )
                nc.vector.tensor_scalar(
                    out=yo[:, half:], in0=ci[:, half:],
                    scalar1=sc_sb[:, 0:1], scalar2=sc_sb[:, 1:2],
                    op0=mybir.AluOpType.mult, op1=mybir.AluOpType.add,
                )
                nc.vector.tensor_scalar_max(out=yo[:, half:], in0=yo[:, half:],
                                            scalar1=0.0)
                nc.sync.dma_start(y_d[o], yo[:])
    nc.compile()
    return nc


# --------------------------------------------------------------------------
# tracing plumbing: make sure the NTFF profile hook exists
# --------------------------------------------------------------------------

def _ensure_trace_hook():
    try:
        import antenv
        try:
            from antenv.axon_hooks import get_axon_ntff_profile_hook
            if get_axon_ntff_profile_hook() is not None:
                return True
        except ImportError:
            import types
            mod = types.ModuleType("antenv.axon_hooks")
            mod._hook = None

            def set_axon_ntff_profile_hook(h):
                mod._hook = h

            def get_axon_ntff_profile_hook():
                return mod._hook

            mod.set_axon_ntff_profile_hook = set_axon_ntff_profile_hook
            mod.get_axon_ntff_profile_hook = get_axon_ntff_profile_hook
            sys.modules["antenv.axon_hooks"] = mod
            antenv.axon_hooks = mod

        # register the ctypes-based hook if libaxon is present
        import contextlib
        import ctypes
        from antenv.axon_hooks import (get_axon_ntff_profile_hook,
                                       set_axon_ntff_profile_hook)
        so_path = "/opt/axon/libaxon_pjrt.so"
        if not os.path.exists(so_path):
            return False
        lib = ctypes.CDLL(so_path)
        if not hasattr(lib, "axon_start_nrt_profile"):
            return False
        lib.axon_start_nrt_profile.argtypes = [
            ctypes.POINTER(ctypes.c_int64), ctypes.c_size_t]
        lib.axon_start_nrt_profile.restype = ctypes.c_int64
        lib.axon_stop_nrt_profile.argtypes = [ctypes.c_char_p]
        lib.axon_stop_nrt_profile.restype = ctypes.c_int64

        @contextlib.contextmanager
        def _hook(output_dir, device_ids):
            import jax
            jax.devices()
            if device_ids:
                ids = (ctypes.c_int64 * len(device_ids))(*device_ids)
                rc = lib.axon_start_nrt_profile(ids, len(device_ids))
            else:
                rc = lib.axon_start_nrt_profile(None, 0)
            if rc != 0:
                raise RuntimeError(f"axon_start_nrt_profile rc={rc}")
            try:
                yield
            finally:
                nf = lib.axon_stop_nrt_profile(str(output_dir).encode())
                if nf < 0:
                    raise RuntimeError(f"axon_stop_nrt_profile rc={nf}")

        set_axon_ntff_profile_hook(_hook)
        return True
    except Exception:
        return False


# --------------------------------------------------------------------------
# entry point
# --------------------------------------------------------------------------

_CACHE = {}


def kernel(features, weights, gamma, beta, neighbor_idx):
    from concourse.bass_utils import run_bass_kernel_spmd

    features = np.asarray(features, dtype=np.float32)
    weights = np.asarray(weights, dtype=np.float32)
    gamma = np.asarray(gamma, dtype=np.float32)
    beta = np.asarray(beta, dtype=np.float32)
    nbr = np.asarray(neighbor_idx, dtype=np.int32)
    n = features.shape[0]

    trace = os.environ.get("KERNEL_TRACE", "1") == "1"
    if trace:
        trace = _ensure_trace_hook()

    geo = _prepare_geometry(nbr)
    G8, mask = _build_planes(geo, features)
    wbk, disp = _build_wblk(geo, weights)
    NP, NV, TFg, COLS, H, BLKP, NOUT = (geo[k] for k in
                                        ("NP", "NV", "TF", "COLS", "H",
                                         "BLKP", "NOUT"))

    key = (NP, NV, TFg, COLS)
    if key not in _CACHE:
        _CACHE[key] = _build_fused(NP, NV, TFg, COLS, H, BLKP, disp)
    nc1 = _CACHE[key]

    total_ns = 0

    # grouping matrices for the on-device BN constant computation
    gmh = np.zeros((128, 16), dtype=np.float32)
    gth = np.zeros((16, 128), dtype=np.float32)
    for p in range(128):
        gmh[p, p % C] = 1.0 / n
        gth[p % C, p] = 1.0
    gbh = np.stack([gamma, beta], axis=1).astype(np.float32)

    in_maps = [{"g8": G8[c * NP:c * NP + NP + 2],
                "msk": mask[c * NP:(c + 1) * NP],
                "wbk": wbk, "gm": gmh, "gt": gth, "gb": gbh}
               for c in range(N_CORES)]
    res1 = run_bass_kernel_spmd(nc1, in_maps, core_ids=list(range(N_CORES)),
                                trace=trace)
    if res1.exec_time_ns is not None:
        total_ns += res1.exec_time_ns

    if total_ns:
        print(f"HW exec time: {total_ns} ns")

    y = np.concatenate([res1.results[c]["y"] for c in range(N_CORES)], axis=0)

    xi, pc = geo["xi"], geo["pc"]
    b = pc // BLKP
    col = pc - b * BLKP
    out = np.empty((n, C), dtype=np.float32)
    for ch in range(C):
        out[:, ch] = y[xi - 1, b * C + ch, col].astype(np.float32)
    return out
